# revision 30
# baseline (speedup 1.0000x reference)
"""Multi-head attention (B=4, P=2048, D=1024, H=16) on 8 TRN2 NeuronCores.

Sharding: tensor-parallel over heads (2 heads per core). Each core computes
qkv for its heads, full attention for its heads, and a partial output
projection (rows of w_proj for its heads). Partials are summed on host.

v7: PE-paced pipeline, every engine under the PE budget per sweep so the
tensor engine never idles (idle gaps reset its p-state and halve its
clock). Per key-block the PE does 2 score + 2 attention-value matmuls
(864ns @2.4GHz); exps alternate between ACT (even blocks, 1147ns) and DVE
fast-exp (odd blocks, 1223ns: affine to int16 bits reinterpreted as bf16),
so each exp engine runs one op per 1728ns budget with slack and no
back-to-back drift. Attention-value matmuls lag scores by 2 blocks to hide
exp latency. Softmax denominators ride a ones-column in the [v|1]
stationary; normalization runs entirely off the PE: ACT copies the
denominator row out of psum, DVE takes a fast reciprocal, the idle GpSimd
engine broadcasts it across partitions, and DVE multiplies straight out of
the attention psum -- scheduled one sweep later so nothing ever waits.
Output projection uses K=128 (both heads' dims stacked on partitions);
V transposes run on the PE (53ns each) from a staging copy.
"""

import numpy as np
import ml_dtypes

import concourse.bass as bass
import concourse.tile as tile
from concourse import bacc, mybir
from concourse import bass_utils
from concourse.masks import make_identity

B, P, D = 4, 2048, 1024
H = 16
NCORES = 8
HPC = H // NCORES          # heads per core = 2
d = D // H                 # 64
R = B * P                  # 8192
SCALE = float(d) ** -0.5

F32 = mybir.dt.float32
BF16 = mybir.dt.bfloat16
I16 = mybir.dt.int16
AF = mybir.ActivationFunctionType
ALU = mybir.AluOpType

# fast-exp: bf16 bits of exp(s*SCALE) ~= round(s*KMUL + BADD)
KMUL = SCALE * float(np.log2(np.e)) * 128.0
BADD = 127.0 * 128.0 - 7.5

_CACHE = {}


def _build():
    nc = bacc.Bacc("TRN2", target_bir_lowering=False, debug=False,
                   enable_asserts=False)
    xT = nc.dram_tensor("xT", (D, R), BF16, kind="ExternalInput").ap()
    wqkv = nc.dram_tensor("wqkv", (128, 3072), BF16, kind="ExternalInput").ap()
    wproj = nc.dram_tensor("wproj", (128, D), BF16, kind="ExternalInput").ap()
    out = nc.dram_tensor("out", (R, D), F32, kind="ExternalOutput").ap()

    xT3 = xT.rearrange("(kb p) n -> p kb n", p=128)      # [128, 8, 8192]
    out3 = out.rearrange("(r p) n -> p r n", p=128)      # [128, 64, 1024]

    with tile.TileContext(nc) as tc:
        from contextlib import ExitStack
        with ExitStack() as ctx:
            p_const = ctx.enter_context(tc.tile_pool(name="const", bufs=1))
            p_w = ctx.enter_context(tc.tile_pool(name="w", bufs=1))
            p_x = ctx.enter_context(tc.tile_pool(name="x", bufs=8))
            p_qk = ctx.enter_context(tc.tile_pool(name="qk", bufs=2))
            p_vt = ctx.enter_context(tc.tile_pool(name="vt", bufs=2))
            p_v = ctx.enter_context(tc.tile_pool(name="v", bufs=2))
            p_e = ctx.enter_context(tc.tile_pool(name="e", bufs=4))
            p_ei = ctx.enter_context(tc.tile_pool(name="ei", bufs=4))
            p_dn = ctx.enter_context(tc.tile_pool(name="dn", bufs=2))
            p_rc = ctx.enter_context(tc.tile_pool(name="rc", bufs=2))
            p_bc = ctx.enter_context(tc.tile_pool(name="bc", bufs=2))
            p_on = ctx.enter_context(tc.tile_pool(name="on", bufs=2))
            p_out = ctx.enter_context(tc.tile_pool(name="o", bufs=3))
            # PSUM: ss 3x[128,1024]f32 (6 banks) + av 2x[*,512] (2) = 8
            ps_ss = ctx.enter_context(
                tc.tile_pool(name="pss", bufs=3, space="PSUM"))
            ps_av = ctx.enter_context(
                tc.tile_pool(name="psav", bufs=2, space="PSUM"))

            ident = p_const.tile([128, 128], BF16)
            make_identity(nc, ident[:])

            wq_sb = p_w.tile([128, 3072], BF16)
            nc.sync.dma_start(wq_sb[:], wqkv[:])
            wp_sb = p_w.tile([128, D], BF16)
            nc.sync.dma_start(wp_sb[:], wproj[:])

            xts = {}

            def emit_x_loads(b):
                for cc in range(4):
                    c = b * 4 + cc
                    xt = p_x.tile([128, 8 * 512], BF16, tag="x")
                    nc.sync.dma_start(
                        xt.rearrange("p (kb n) -> p kb n", n=512),
                        xT3[:, :, c * 512:(c + 1) * 512])
                    xts[(b, cc)] = xt

            def emit_proj_unit(b, rr, oTn):
                """one token-block of the output projection: 2 matmuls
                into one [128,1024] tile of the 3-deep ss-ring, one wide
                copy (alternating ACT/DVE) + store"""
                psP = ps_ss.tile([128, 1024], F32, tag="ss", name="psP")
                nc.tensor.matmul(psP[:, 0:512],
                                 oTn[:, rr * 128:(rr + 1) * 128],
                                 wp_sb[:, 0:512], start=True, stop=True)
                nc.tensor.matmul(psP[:, 512:1024],
                                 oTn[:, rr * 128:(rr + 1) * 128],
                                 wp_sb[:, 512:1024], start=True, stop=True)
                outsb = p_out.tile([128, 1024], F32, tag="os")
                if rr % 2 == 0:
                    nc.scalar.copy(outsb[:], psP[:])
                else:
                    nc.vector.tensor_copy(outsb[:], psP[:])
                r0 = b * 16 + rr
                nc.sync.dma_start(
                    out3[:, r0:r0 + 1, :],
                    outsb.rearrange("p (r n) -> p r n", n=1024))

            def emit_stage_a(b, proj_prev=None):
                """qkv for batch b -> qt, kt, vON2. Interleaves the previous
                batch's output projection (4 token-blocks per cc) so its
                psum copies spread across the stage instead of jamming the
                ACT/DVE queues at a phase boundary."""
                qt = p_qk.tile([128, P], BF16, tag="qt", name=f"qt{b}")
                kt = p_qk.tile([128, P], BF16, tag="kt", name=f"kt{b}")
                vON2 = p_v.tile([128, 16 * 130], BF16, tag="v",
                                name=f"vON{b}")
                vv = vON2.rearrange("p (blk w) -> p blk w", w=130)
                nc.vector.memset(vv[:, :, 64:65], 1.0)
                nc.vector.memset(vv[:, :, 129:130], 1.0)
                for cc in range(4):
                    xt = xts.pop((b, cc))
                    # v first: its staging copy drains while the q/k matmuls
                    # run, so the transposes at the end never wait
                    psB = ps_ss.tile([128, 1024], F32, tag="ss", name="psB")
                    for kb in range(8):
                        col = kb * 384 + 256
                        nc.tensor.matmul(
                            psB[:, 0:512], wq_sb[:, col:col + 128],
                            xt[:, kb * 512:(kb + 1) * 512],
                            start=(kb == 0), stop=(kb == 7))
                    vtmp = p_vt.tile([128, 512], BF16, tag="vt")
                    nc.vector.tensor_copy(vtmp[:], psB[:, 0:512])
                    psA = ps_ss.tile([128, 1024], F32, tag="ss", name="psA")
                    for m in range(2):
                        for kb in range(8):
                            col = kb * 384 + m * 128
                            nc.tensor.matmul(
                                psA[:, m * 512:(m + 1) * 512],
                                wq_sb[:, col:col + 128],
                                xt[:, kb * 512:(kb + 1) * 512],
                                start=(kb == 0), stop=(kb == 7))
                    psT = ps_av.tile([128, 512], BF16, tag="av", name="psT")
                    for i in range(4):
                        nc.tensor.transpose(
                            psT[:, i * 128:(i + 1) * 128],
                            vtmp[:, i * 128:(i + 1) * 128], ident[:])
                    nc.vector.tensor_copy(
                        qt[:, cc * 512:(cc + 1) * 512], psA[:, 0:512])
                    nc.vector.tensor_copy(
                        kt[:, cc * 512:(cc + 1) * 512], psA[:, 512:1024])
                    for i in range(4):
                        jb = cc * 4 + i
                        dstv = vv[:, jb, 0:130].rearrange(
                            "p (two s) -> p two s", s=65)[:, :, 0:64]
                        srcv = psT[:, i * 128:(i + 1) * 128].rearrange(
                            "p (two s) -> p two s", s=64)
                        nc.vector.tensor_copy(dstv, srcv)
                    if proj_prev is not None:
                        pb_, oTn_ = proj_prev
                        for rr in range(cc * 4, cc * 4 + 4):
                            emit_proj_unit(pb_, rr, oTn_)
                return qt, kt, vON2

            def emit_sweep(qt, kt, vON2, oTn, h, qc, filler=None, drain=False):
                """scores+exp+av for one (head, 1024-query chunk).

                Returns a closure finishing this sweep's normalization; the
                caller passes it as `filler` into the NEXT sweep (emitted
                after key-block 3) so every input is long since ready.
                """
                q0 = qc * 1024
                avps = [ps_av.tile([65, 512], F32, tag="av", name="avps")
                        for _ in range(2)]
                vs = vON2.rearrange("p (blk w) -> p blk w", w=130)
                pend = []

                def emit_av(jb, e_ap):
                    for half in range(2):
                        nc.tensor.matmul(
                            avps[half][:],
                            vs[:, jb, h * 65:(h + 1) * 65],
                            e_ap[:, half * 512:(half + 1) * 512],
                            start=(jb == 0), stop=(jb == 15))

                for jb in range(16):
                    pss = ps_ss.tile([128, 1024], F32, tag="ss", name="pss")
                    for half in range(2):
                        nc.tensor.matmul(
                            pss[:, half * 512:(half + 1) * 512],
                            kt[h * 64:(h + 1) * 64, jb * 128:(jb + 1) * 128],
                            qt[h * 64:(h + 1) * 64,
                               q0 + half * 512:q0 + (half + 1) * 512],
                            start=True, stop=True)
                    if jb % 2 == 1:
                        ei = p_ei.tile([128, 1024], I16, tag="ei")
                        nc.vector.tensor_scalar(ei[:], pss[:], KMUL, BADD,
                                                ALU.mult, ALU.add)
                        e_ap = ei[:].bitcast(BF16)
                    else:
                        et = p_e.tile([128, 1024], BF16, tag="e")
                        nc.scalar.activation(et[:], pss[:], AF.Exp,
                                             scale=SCALE)
                        e_ap = et[:]
                    pend.append((jb, e_ap))
                    if len(pend) > 3:
                        emit_av(*pend.pop(0))
                    if jb in (3, 7, 11) and filler is not None:
                        filler(jb // 4)
                for item in pend:
                    emit_av(*item)

                # denominator row to partition 0: psum -> sbuf same-lane,
                # then an sbuf->sbuf copy with a 64->0 partition shift (ACT
                # has slack in every sweep)
                den0 = p_dn.tile([1, 1024], F32, tag="d0")
                if drain:
                    # last sweep of the batch: move o^T out of psum right
                    # away (on ACT -- DVE must stay clear for the upcoming
                    # norm + stage-A copies) so projection reuses the banks
                    oTu = p_rc.tile([65, 1024], F32, tag="u")
                    for half in range(2):
                        nc.scalar.copy(
                            oTu[:, half * 512:(half + 1) * 512],
                            avps[half][:])
                    nc.scalar.copy(den0[0:1, :], oTu[64:65, :])
                    srcs = [oTu[0:64, 0:512], oTu[0:64, 512:1024]]
                else:
                    den = p_dn.tile([65, 1024], F32, tag="dn")
                    for half in range(2):
                        nc.scalar.copy(
                            den[64:65, half * 512:(half + 1) * 512],
                            avps[half][64:65, :])
                    nc.scalar.copy(den0[0:1, :], den[64:65, :])

                state = {}

                def norm_step(step):
                    # staged so the DVE filler ops interleave between
                    # fast-exps instead of jamming the queue mid-sweep
                    if step == 0:
                        rcp = p_dn.tile([1, 1024], F32, tag="rc")
                        nc.vector.reciprocal_approx_fast(
                            rcp[0:1, :], den0[0:1, :])
                        bcs = p_bc.tile([64, 1024], F32, tag="bc")
                        nc.gpsimd.partition_broadcast(bcs[:], rcp[0:1, :])
                        state["bcs"] = bcs
                    else:
                        half = step - 1
                        src = srcs[half] if drain else avps[half][0:64, :]
                        nc.vector.tensor_mul(
                            oTn[h * 64:(h + 1) * 64,
                                q0 + half * 512:q0 + (half + 1) * 512],
                            src,
                            state["bcs"][:, half * 512:(half + 1) * 512])
                return norm_step

            emit_x_loads(0)
            proj_prev = None
            for b in range(B):
                qt, kt, vON2 = emit_stage_a(b, proj_prev)
                if b + 1 < B:
                    emit_x_loads(b + 1)
                oTn = p_on.tile([128, P], BF16, tag="on", name=f"oTn{b}")
                fn = None
                for si, (h, qc) in enumerate([(0, 0), (0, 1), (1, 0), (1, 1)]):
                    fn = emit_sweep(qt, kt, vON2, oTn, h, qc, filler=fn,
                                    drain=True)
                for stp in range(3):
                    fn(stp)
                proj_prev = (b, oTn)
            # last batch's projection has no following stage A
            for rr in range(16):
                emit_proj_unit(B - 1, rr, proj_prev[1])

    nc.compile()
    return nc


def _in_maps(x, w_qkv, w_proj):
    x2 = np.ascontiguousarray(x.reshape(R, D).T)          # (D, R)
    xbf = x2.astype(ml_dtypes.bfloat16)
    Wq = w_qkv.reshape(D, 3, H, d)
    Wp = w_proj.reshape(H, d, D)
    maps = []
    for c in range(NCORES):
        hs = slice(c * HPC, (c + 1) * HPC)
        # per-core qkv weight shard, columns ordered (qkv, head, d)
        w_shard = np.ascontiguousarray(Wq[:, :, hs, :]).reshape(D, 3 * HPC * d)
        # pre-tile: [p, kb*384 + m*128 + col] = w_shard[kb*128+p, m*128+col]
        wq_pre = np.ascontiguousarray(
            w_shard.reshape(8, 128, 3, 128).transpose(1, 0, 2, 3)
        ).reshape(128, 3072)
        wp_shard = np.ascontiguousarray(Wp[hs]).reshape(HPC * d, D)
        maps.append({
            "xT": xbf,
            "wqkv": np.ascontiguousarray(wq_pre).astype(ml_dtypes.bfloat16),
            "wproj": wp_shard.astype(ml_dtypes.bfloat16),
        })
    return maps


def get_nc():
    if "nc" not in _CACHE:
        _CACHE["nc"] = _build()
    return _CACHE["nc"]


def kernel(x, w_qkv, w_proj, b_proj):
    x = np.asarray(x)
    w_qkv = np.asarray(w_qkv)
    w_proj = np.asarray(w_proj)
    b_proj = np.asarray(b_proj)
    nc = get_nc()
    maps = _in_maps(x, w_qkv, w_proj)
    res = bass_utils.run_bass_kernel_spmd(nc, maps, core_ids=list(range(NCORES)))
    acc = np.zeros((R, D), dtype=np.float64)
    for r in res.results:
        acc += r["out"].astype(np.float64)
    acc += b_proj.astype(np.float64)
    return acc.reshape(B, P, D).astype(np.float32)


# revision 31
# speedup vs baseline: 1.1073x; 1.1073x over previous
"""Multi-head attention (B=4, P=2048, D=1024, H=16) on 8 TRN2 NeuronCores.

Sharding: tensor-parallel over heads (2 heads per core). Each core computes
qkv for its heads, full attention for its heads, and a partial output
projection (rows of w_proj for its heads). Partials are summed on host.

v7: PE-paced pipeline, every engine under the PE budget per sweep so the
tensor engine never idles (idle gaps reset its p-state and halve its
clock). Per key-block the PE does 2 score + 2 attention-value matmuls
(864ns @2.4GHz); exps alternate between ACT (even blocks, 1147ns) and DVE
fast-exp (odd blocks, 1223ns: affine to int16 bits reinterpreted as bf16),
so each exp engine runs one op per 1728ns budget with slack and no
back-to-back drift. Attention-value matmuls lag scores by 2 blocks to hide
exp latency. Softmax denominators ride a ones-column in the [v|1]
stationary; normalization runs entirely off the PE: ACT copies the
denominator row out of psum, DVE takes a fast reciprocal, the idle GpSimd
engine broadcasts it across partitions, and DVE multiplies straight out of
the attention psum -- scheduled one sweep later so nothing ever waits.
Output projection uses K=128 (both heads' dims stacked on partitions);
V transposes run on the PE (53ns each) from a staging copy.
"""

import numpy as np
import ml_dtypes

import concourse.bass as bass
import concourse.tile as tile
from concourse import bacc, mybir
from concourse import bass_utils
from concourse.masks import make_identity

B, P, D = 4, 2048, 1024
H = 16
NCORES = 8
HPC = H // NCORES          # heads per core = 2
d = D // H                 # 64
R = B * P                  # 8192
SCALE = float(d) ** -0.5

F32 = mybir.dt.float32
BF16 = mybir.dt.bfloat16
I16 = mybir.dt.int16
AF = mybir.ActivationFunctionType
ALU = mybir.AluOpType

# fast-exp: bf16 bits of exp(s*SCALE) ~= round(s*KMUL + BADD)
KMUL = SCALE * float(np.log2(np.e)) * 128.0
BADD = 127.0 * 128.0 - 7.5

_CACHE = {}


def _build():
    nc = bacc.Bacc("TRN2", target_bir_lowering=False, debug=False,
                   enable_asserts=False)
    xT = nc.dram_tensor("xT", (D, R), BF16, kind="ExternalInput").ap()
    wqkv = nc.dram_tensor("wqkv", (128, 3072), BF16, kind="ExternalInput").ap()
    wproj = nc.dram_tensor("wproj", (128, D), BF16, kind="ExternalInput").ap()
    out = nc.dram_tensor("out", (R, D), F32, kind="ExternalOutput").ap()

    xT3 = xT.rearrange("(kb p) n -> p kb n", p=128)      # [128, 8, 8192]
    out3 = out.rearrange("(r p) n -> p r n", p=128)      # [128, 64, 1024]

    with tile.TileContext(nc) as tc:
        from contextlib import ExitStack
        with ExitStack() as ctx:
            p_const = ctx.enter_context(tc.tile_pool(name="const", bufs=1))
            p_w = ctx.enter_context(tc.tile_pool(name="w", bufs=1))
            p_x = ctx.enter_context(tc.tile_pool(name="x", bufs=8))
            p_qk = ctx.enter_context(tc.tile_pool(name="qk", bufs=2))
            p_vt = ctx.enter_context(tc.tile_pool(name="vt", bufs=2))
            p_v = ctx.enter_context(tc.tile_pool(name="v", bufs=2))
            p_e = ctx.enter_context(tc.tile_pool(name="e", bufs=4))
            p_ei = ctx.enter_context(tc.tile_pool(name="ei", bufs=4))
            p_dn = ctx.enter_context(tc.tile_pool(name="dn", bufs=2))
            p_rc = ctx.enter_context(tc.tile_pool(name="rc", bufs=2))
            p_bc = ctx.enter_context(tc.tile_pool(name="bc", bufs=2))
            p_on = ctx.enter_context(tc.tile_pool(name="on", bufs=2))
            p_out = ctx.enter_context(tc.tile_pool(name="o", bufs=3))
            # PSUM: ss 2x[128,1024]f32 (4 banks) + av 4x[*,512] (4) = 8
            ps_ss = ctx.enter_context(
                tc.tile_pool(name="pss", bufs=2, space="PSUM"))
            ps_av = ctx.enter_context(
                tc.tile_pool(name="psav", bufs=4, space="PSUM"))

            ident = p_const.tile([128, 128], BF16)
            make_identity(nc, ident[:])

            wq_sb = p_w.tile([128, 3072], BF16)
            nc.sync.dma_start(wq_sb[:], wqkv[:])
            wp_sb = p_w.tile([128, D], BF16)
            nc.sync.dma_start(wp_sb[:], wproj[:])

            xts = {}

            def emit_x_loads(b):
                for cc in range(4):
                    c = b * 4 + cc
                    xt = p_x.tile([128, 8 * 512], BF16, tag="x")
                    nc.sync.dma_start(
                        xt.rearrange("p (kb n) -> p kb n", n=512),
                        xT3[:, :, c * 512:(c + 1) * 512])
                    xts[(b, cc)] = xt

            def emit_proj_unit(b, rr, oTn):
                """one token-block of the output projection (2 matmuls into
                the [128,512] av-ring + copies split ACT/DVE + store)"""
                pa = ps_av.tile([128, 512], F32, tag="av", name="pPa")
                pb = ps_av.tile([128, 512], F32, tag="av", name="pPb")
                nc.tensor.matmul(pa[:], oTn[:, rr * 128:(rr + 1) * 128],
                                 wp_sb[:, 0:512], start=True, stop=True)
                nc.tensor.matmul(pb[:], oTn[:, rr * 128:(rr + 1) * 128],
                                 wp_sb[:, 512:1024], start=True, stop=True)
                outsb = p_out.tile([128, 1024], F32, tag="os")
                if rr % 2 == 0:
                    nc.scalar.copy(outsb[:, 0:512], pa[:])
                    nc.scalar.copy(outsb[:, 512:1024], pb[:])
                else:
                    nc.vector.tensor_copy(outsb[:, 0:512], pa[:])
                    nc.vector.tensor_copy(outsb[:, 512:1024], pb[:])
                r0 = b * 16 + rr
                nc.sync.dma_start(
                    out3[:, r0:r0 + 1, :],
                    outsb.rearrange("p (r n) -> p r n", n=1024))

            def emit_stage_a(b, proj_prev=None):
                """qkv for batch b -> qt, kt, vON2. Interleaves the previous
                batch's output projection (4 token-blocks per cc) so its
                psum copies spread across the stage instead of jamming the
                ACT/DVE queues at a phase boundary."""
                qt = p_qk.tile([128, P], BF16, tag="qt", name=f"qt{b}")
                kt = p_qk.tile([128, P], BF16, tag="kt", name=f"kt{b}")
                vON2 = p_v.tile([128, 16 * 130], BF16, tag="v",
                                name=f"vON{b}")
                vv = vON2.rearrange("p (blk w) -> p blk w", w=130)
                nc.vector.memset(vv[:, :, 64:65], 1.0)
                nc.vector.memset(vv[:, :, 129:130], 1.0)
                for cc in range(4):
                    xt = xts.pop((b, cc))
                    # v first: its staging copy drains while the q/k matmuls
                    # run, so the transposes at the end never wait
                    psB = ps_ss.tile([128, 1024], F32, tag="ss", name="psB")
                    for kb in range(8):
                        col = kb * 384 + 256
                        nc.tensor.matmul(
                            psB[:, 0:512], wq_sb[:, col:col + 128],
                            xt[:, kb * 512:(kb + 1) * 512],
                            start=(kb == 0), stop=(kb == 7))
                    vtmp = p_vt.tile([128, 512], BF16, tag="vt")
                    nc.vector.tensor_copy(vtmp[:], psB[:, 0:512])
                    psA = ps_ss.tile([128, 1024], F32, tag="ss", name="psA")
                    for m in range(2):
                        for kb in range(8):
                            col = kb * 384 + m * 128
                            nc.tensor.matmul(
                                psA[:, m * 512:(m + 1) * 512],
                                wq_sb[:, col:col + 128],
                                xt[:, kb * 512:(kb + 1) * 512],
                                start=(kb == 0), stop=(kb == 7))
                    psT = ps_av.tile([128, 512], BF16, tag="av", name="psT")
                    for i in range(4):
                        nc.tensor.transpose(
                            psT[:, i * 128:(i + 1) * 128],
                            vtmp[:, i * 128:(i + 1) * 128], ident[:])
                    nc.vector.tensor_copy(
                        qt[:, cc * 512:(cc + 1) * 512], psA[:, 0:512])
                    nc.vector.tensor_copy(
                        kt[:, cc * 512:(cc + 1) * 512], psA[:, 512:1024])
                    for i in range(4):
                        jb = cc * 4 + i
                        dstv = vv[:, jb, 0:130].rearrange(
                            "p (two s) -> p two s", s=65)[:, :, 0:64]
                        srcv = psT[:, i * 128:(i + 1) * 128].rearrange(
                            "p (two s) -> p two s", s=64)
                        nc.vector.tensor_copy(dstv, srcv)
                    if proj_prev is not None:
                        pb_, oTn_ = proj_prev
                        for rr in range(cc * 4, cc * 4 + 4):
                            emit_proj_unit(pb_, rr, oTn_)
                return qt, kt, vON2

            def emit_sweep(qt, kt, vON2, oTn, h, qc, filler=None, drain=False):
                """scores+exp+av for one (head, 1024-query chunk).

                Returns a closure finishing this sweep's normalization; the
                caller passes it as `filler` into the NEXT sweep (emitted
                after key-block 3) so every input is long since ready.
                """
                q0 = qc * 1024
                avps = [ps_av.tile([65, 512], F32, tag="av", name="avps")
                        for _ in range(2)]
                vs = vON2.rearrange("p (blk w) -> p blk w", w=130)
                pend = []

                def emit_av(jb, e_ap):
                    for half in range(2):
                        nc.tensor.matmul(
                            avps[half][:],
                            vs[:, jb, h * 65:(h + 1) * 65],
                            e_ap[:, half * 512:(half + 1) * 512],
                            start=(jb == 0), stop=(jb == 15))

                for jb in range(16):
                    pss = ps_ss.tile([128, 1024], F32, tag="ss", name="pss")
                    for half in range(2):
                        nc.tensor.matmul(
                            pss[:, half * 512:(half + 1) * 512],
                            kt[h * 64:(h + 1) * 64, jb * 128:(jb + 1) * 128],
                            qt[h * 64:(h + 1) * 64,
                               q0 + half * 512:q0 + (half + 1) * 512],
                            start=True, stop=True)
                    if jb % 2 == 1:
                        ei = p_ei.tile([128, 1024], I16, tag="ei")
                        nc.vector.tensor_scalar(ei[:], pss[:], KMUL, BADD,
                                                ALU.mult, ALU.add)
                        e_ap = ei[:].bitcast(BF16)
                    else:
                        et = p_e.tile([128, 1024], BF16, tag="e")
                        nc.scalar.activation(et[:], pss[:], AF.Exp,
                                             scale=SCALE)
                        e_ap = et[:]
                    pend.append((jb, e_ap))
                    if len(pend) > 3:
                        emit_av(*pend.pop(0))
                    if jb in (3, 7, 11) and filler is not None:
                        filler(jb // 4)
                for item in pend:
                    emit_av(*item)

                # denominator row to partition 0: psum -> sbuf same-lane,
                # then an sbuf->sbuf copy with a 64->0 partition shift (ACT
                # has slack in every sweep)
                den0 = p_dn.tile([1, 1024], F32, tag="d0")
                if drain:
                    # last sweep of the batch: move o^T out of psum right
                    # away (on ACT -- DVE must stay clear for the upcoming
                    # norm + stage-A copies) so projection reuses the banks
                    oTu = p_rc.tile([65, 1024], F32, tag="u")
                    for half in range(2):
                        nc.scalar.copy(
                            oTu[:, half * 512:(half + 1) * 512],
                            avps[half][:])
                    nc.scalar.copy(den0[0:1, :], oTu[64:65, :])
                    srcs = [oTu[0:64, 0:512], oTu[0:64, 512:1024]]
                else:
                    den = p_dn.tile([65, 1024], F32, tag="dn")
                    for half in range(2):
                        nc.scalar.copy(
                            den[64:65, half * 512:(half + 1) * 512],
                            avps[half][64:65, :])
                    nc.scalar.copy(den0[0:1, :], den[64:65, :])

                state = {}

                def norm_step(step):
                    # staged so the DVE filler ops interleave between
                    # fast-exps instead of jamming the queue mid-sweep
                    if step == 0:
                        rcp = p_dn.tile([1, 1024], F32, tag="rc")
                        nc.vector.reciprocal_approx_fast(
                            rcp[0:1, :], den0[0:1, :])
                        bcs = p_bc.tile([64, 1024], F32, tag="bc")
                        nc.gpsimd.partition_broadcast(bcs[:], rcp[0:1, :])
                        state["bcs"] = bcs
                    else:
                        half = step - 1
                        src = srcs[half] if drain else avps[half][0:64, :]
                        nc.vector.tensor_mul(
                            oTn[h * 64:(h + 1) * 64,
                                q0 + half * 512:q0 + (half + 1) * 512],
                            src,
                            state["bcs"][:, half * 512:(half + 1) * 512])
                return norm_step

            emit_x_loads(0)
            proj_prev = None
            for b in range(B):
                qt, kt, vON2 = emit_stage_a(b, proj_prev)
                if b + 1 < B:
                    emit_x_loads(b + 1)
                oTn = p_on.tile([128, P], BF16, tag="on", name=f"oTn{b}")
                fn = None
                for si, (h, qc) in enumerate([(0, 0), (0, 1), (1, 0), (1, 1)]):
                    fn = emit_sweep(qt, kt, vON2, oTn, h, qc, filler=fn,
                                    drain=(si == 3))
                for stp in range(3):
                    fn(stp)
                proj_prev = (b, oTn)
            # last batch's projection has no following stage A
            for rr in range(16):
                emit_proj_unit(B - 1, rr, proj_prev[1])

    nc.compile()
    return nc


def _in_maps(x, w_qkv, w_proj):
    x2 = np.ascontiguousarray(x.reshape(R, D).T)          # (D, R)
    xbf = x2.astype(ml_dtypes.bfloat16)
    Wq = w_qkv.reshape(D, 3, H, d)
    Wp = w_proj.reshape(H, d, D)
    maps = []
    for c in range(NCORES):
        hs = slice(c * HPC, (c + 1) * HPC)
        # per-core qkv weight shard, columns ordered (qkv, head, d)
        w_shard = np.ascontiguousarray(Wq[:, :, hs, :]).reshape(D, 3 * HPC * d)
        # pre-tile: [p, kb*384 + m*128 + col] = w_shard[kb*128+p, m*128+col]
        wq_pre = np.ascontiguousarray(
            w_shard.reshape(8, 128, 3, 128).transpose(1, 0, 2, 3)
        ).reshape(128, 3072)
        wp_shard = np.ascontiguousarray(Wp[hs]).reshape(HPC * d, D)
        maps.append({
            "xT": xbf,
            "wqkv": np.ascontiguousarray(wq_pre).astype(ml_dtypes.bfloat16),
            "wproj": wp_shard.astype(ml_dtypes.bfloat16),
        })
    return maps


def get_nc():
    if "nc" not in _CACHE:
        _CACHE["nc"] = _build()
    return _CACHE["nc"]


def kernel(x, w_qkv, w_proj, b_proj):
    x = np.asarray(x)
    w_qkv = np.asarray(w_qkv)
    w_proj = np.asarray(w_proj)
    b_proj = np.asarray(b_proj)
    nc = get_nc()
    maps = _in_maps(x, w_qkv, w_proj)
    res = bass_utils.run_bass_kernel_spmd(nc, maps, core_ids=list(range(NCORES)))
    acc = np.zeros((R, D), dtype=np.float64)
    for r in res.results:
        acc += r["out"].astype(np.float64)
    acc += b_proj.astype(np.float64)
    return acc.reshape(B, P, D).astype(np.float32)


# revision 32
# speedup vs baseline: 1.1110x; 1.0034x over previous
"""Multi-head attention (B=4, P=2048, D=1024, H=16) on 8 TRN2 NeuronCores.

Sharding: tensor-parallel over heads (2 heads per core). Each core computes
qkv for its heads, full attention for its heads, and a partial output
projection (rows of w_proj for its heads). Partials are summed on host.

v7: PE-paced pipeline, every engine under the PE budget per sweep so the
tensor engine never idles (idle gaps reset its p-state and halve its
clock). Per key-block the PE does 2 score + 2 attention-value matmuls
(864ns @2.4GHz); exps alternate between ACT (even blocks, 1147ns) and DVE
fast-exp (odd blocks, 1223ns: affine to int16 bits reinterpreted as bf16),
so each exp engine runs one op per 1728ns budget with slack and no
back-to-back drift. Attention-value matmuls lag scores by 2 blocks to hide
exp latency. Softmax denominators ride a ones-column in the [v|1]
stationary; normalization runs entirely off the PE: ACT copies the
denominator row out of psum, DVE takes a fast reciprocal, the idle GpSimd
engine broadcasts it across partitions, and DVE multiplies straight out of
the attention psum -- scheduled one sweep later so nothing ever waits.
Output projection uses K=128 (both heads' dims stacked on partitions);
V transposes run on the PE (53ns each) from a staging copy.
"""

import numpy as np
import ml_dtypes

import concourse.bass as bass
import concourse.tile as tile
from concourse import bacc, mybir
from concourse import bass_utils
from concourse.masks import make_identity

B, P, D = 4, 2048, 1024
H = 16
NCORES = 8
HPC = H // NCORES          # heads per core = 2
d = D // H                 # 64
R = B * P                  # 8192
SCALE = float(d) ** -0.5

F32 = mybir.dt.float32
BF16 = mybir.dt.bfloat16
I16 = mybir.dt.int16
AF = mybir.ActivationFunctionType
ALU = mybir.AluOpType

# fast-exp: bf16 bits of exp(s*SCALE) ~= round(s*KMUL + BADD)
KMUL = SCALE * float(np.log2(np.e)) * 128.0
BADD = 127.0 * 128.0 - 7.5

_CACHE = {}


def _build():
    nc = bacc.Bacc("TRN2", target_bir_lowering=False, debug=False,
                   enable_asserts=False)
    xT = nc.dram_tensor("xT", (128, 16 * 8 * 512), BF16,
                        kind="ExternalInput").ap()
    wqkv = nc.dram_tensor("wqkv", (128, 3072), BF16, kind="ExternalInput").ap()
    wproj = nc.dram_tensor("wproj", (128, D), BF16, kind="ExternalInput").ap()
    out = nc.dram_tensor("out", (R, D), F32, kind="ExternalOutput").ap()

    xT4 = xT.rearrange("p (c kb n) -> p c kb n", kb=8, n=512)
    out3 = out.rearrange("(r p) n -> p r n", p=128)      # [128, 64, 1024]

    with tile.TileContext(nc) as tc:
        from contextlib import ExitStack
        with ExitStack() as ctx:
            p_const = ctx.enter_context(tc.tile_pool(name="const", bufs=1))
            p_w = ctx.enter_context(tc.tile_pool(name="w", bufs=1))
            p_x = ctx.enter_context(tc.tile_pool(name="x", bufs=8))
            p_qk = ctx.enter_context(tc.tile_pool(name="qk", bufs=2))
            p_vt = ctx.enter_context(tc.tile_pool(name="vt", bufs=2))
            p_v = ctx.enter_context(tc.tile_pool(name="v", bufs=2))
            p_e = ctx.enter_context(tc.tile_pool(name="e", bufs=4))
            p_ei = ctx.enter_context(tc.tile_pool(name="ei", bufs=4))
            p_dn = ctx.enter_context(tc.tile_pool(name="dn", bufs=2))
            p_rc = ctx.enter_context(tc.tile_pool(name="rc", bufs=2))
            p_bc = ctx.enter_context(tc.tile_pool(name="bc", bufs=2))
            p_on = ctx.enter_context(tc.tile_pool(name="on", bufs=2))
            p_out = ctx.enter_context(tc.tile_pool(name="o", bufs=3))
            # PSUM: ss 2x[128,1024]f32 (4 banks) + av 4x[*,512] (4) = 8
            ps_ss = ctx.enter_context(
                tc.tile_pool(name="pss", bufs=2, space="PSUM"))
            ps_av = ctx.enter_context(
                tc.tile_pool(name="psav", bufs=4, space="PSUM"))

            ident = p_const.tile([128, 128], BF16)
            make_identity(nc, ident[:])

            xts = {}

            def emit_x_loads(b):
                for cc in range(4):
                    c = b * 4 + cc
                    xt = p_x.tile([128, 8 * 512], BF16, tag="x")
                    nc.sync.dma_start(
                        xt.rearrange("p (kb n) -> p kb n", n=512),
                        xT4[:, c, :, :])
                    xts[(b, cc)] = xt

            def emit_proj_unit(b, rr, oTn):
                """one token-block of the output projection (2 matmuls into
                the [128,512] av-ring + copies split ACT/DVE + store)"""
                pa = ps_av.tile([128, 512], F32, tag="av", name="pPa")
                pb = ps_av.tile([128, 512], F32, tag="av", name="pPb")
                nc.tensor.matmul(pa[:], oTn[:, rr * 128:(rr + 1) * 128],
                                 wp_sb[:, 0:512], start=True, stop=True)
                nc.tensor.matmul(pb[:], oTn[:, rr * 128:(rr + 1) * 128],
                                 wp_sb[:, 512:1024], start=True, stop=True)
                outsb = p_out.tile([128, 1024], F32, tag="os")
                if rr % 2 == 0:
                    nc.scalar.copy(outsb[:, 0:512], pa[:])
                    nc.scalar.copy(outsb[:, 512:1024], pb[:])
                else:
                    nc.vector.tensor_copy(outsb[:, 0:512], pa[:])
                    nc.vector.tensor_copy(outsb[:, 512:1024], pb[:])
                r0 = b * 16 + rr
                nc.sync.dma_start(
                    out3[:, r0:r0 + 1, :],
                    outsb.rearrange("p (r n) -> p r n", n=1024))

            def emit_stage_a(b, proj_prev=None):
                """qkv for batch b -> qt, kt, vON2. Interleaves the previous
                batch's output projection (4 token-blocks per cc) so its
                psum copies spread across the stage instead of jamming the
                ACT/DVE queues at a phase boundary."""
                qt = p_qk.tile([128, P], BF16, tag="qt", name=f"qt{b}")
                kt = p_qk.tile([128, P], BF16, tag="kt", name=f"kt{b}")
                vON2 = p_v.tile([128, 16 * 130], BF16, tag="v",
                                name=f"vON{b}")
                vv = vON2.rearrange("p (blk w) -> p blk w", w=130)
                nc.vector.memset(vv[:, :, 64:65], 1.0)
                nc.vector.memset(vv[:, :, 129:130], 1.0)
                for cc in range(4):
                    xt = xts.pop((b, cc))
                    # v first: its staging copy drains while the q/k matmuls
                    # run, so the transposes at the end never wait
                    psB = ps_ss.tile([128, 1024], F32, tag="ss", name="psB")
                    for kb in range(8):
                        col = kb * 384 + 256
                        nc.tensor.matmul(
                            psB[:, 0:512], wq_sb[:, col:col + 128],
                            xt[:, kb * 512:(kb + 1) * 512],
                            start=(kb == 0), stop=(kb == 7))
                    vtmp = p_vt.tile([128, 512], BF16, tag="vt")
                    nc.vector.tensor_copy(vtmp[:], psB[:, 0:512])
                    psA = ps_ss.tile([128, 1024], F32, tag="ss", name="psA")
                    for m in range(2):
                        for kb in range(8):
                            col = kb * 384 + m * 128
                            nc.tensor.matmul(
                                psA[:, m * 512:(m + 1) * 512],
                                wq_sb[:, col:col + 128],
                                xt[:, kb * 512:(kb + 1) * 512],
                                start=(kb == 0), stop=(kb == 7))
                    psT = ps_av.tile([128, 512], BF16, tag="av", name="psT")
                    for i in range(4):
                        nc.tensor.transpose(
                            psT[:, i * 128:(i + 1) * 128],
                            vtmp[:, i * 128:(i + 1) * 128], ident[:])
                    nc.vector.tensor_copy(
                        qt[:, cc * 512:(cc + 1) * 512], psA[:, 0:512])
                    nc.vector.tensor_copy(
                        kt[:, cc * 512:(cc + 1) * 512], psA[:, 512:1024])
                    for i in range(4):
                        jb = cc * 4 + i
                        dstv = vv[:, jb, 0:130].rearrange(
                            "p (two s) -> p two s", s=65)[:, :, 0:64]
                        srcv = psT[:, i * 128:(i + 1) * 128].rearrange(
                            "p (two s) -> p two s", s=64)
                        nc.vector.tensor_copy(dstv, srcv)
                    if proj_prev is not None:
                        pb_, oTn_ = proj_prev
                        for rr in range(cc * 4, cc * 4 + 4):
                            emit_proj_unit(pb_, rr, oTn_)
                return qt, kt, vON2

            def emit_sweep(qt, kt, vON2, oTn, h, qc, filler=None, drain=False):
                """scores+exp+av for one (head, 1024-query chunk).

                Returns a closure finishing this sweep's normalization; the
                caller passes it as `filler` into the NEXT sweep (emitted
                after key-block 3) so every input is long since ready.
                """
                q0 = qc * 1024
                avps = [ps_av.tile([65, 512], F32, tag="av", name="avps")
                        for _ in range(2)]
                vs = vON2.rearrange("p (blk w) -> p blk w", w=130)
                pend = []

                def emit_av(jb, e_ap):
                    for half in range(2):
                        nc.tensor.matmul(
                            avps[half][:],
                            vs[:, jb, h * 65:(h + 1) * 65],
                            e_ap[:, half * 512:(half + 1) * 512],
                            start=(jb == 0), stop=(jb == 15))

                for jb in range(16):
                    pss = ps_ss.tile([128, 1024], F32, tag="ss", name="pss")
                    for half in range(2):
                        nc.tensor.matmul(
                            pss[:, half * 512:(half + 1) * 512],
                            kt[h * 64:(h + 1) * 64, jb * 128:(jb + 1) * 128],
                            qt[h * 64:(h + 1) * 64,
                               q0 + half * 512:q0 + (half + 1) * 512],
                            start=True, stop=True)
                    if jb % 2 == 1:
                        ei = p_ei.tile([128, 1024], I16, tag="ei")
                        nc.vector.tensor_scalar(ei[:], pss[:], KMUL, BADD,
                                                ALU.mult, ALU.add)
                        e_ap = ei[:].bitcast(BF16)
                    else:
                        et = p_e.tile([128, 1024], BF16, tag="e")
                        nc.scalar.activation(et[:], pss[:], AF.Exp,
                                             scale=SCALE)
                        e_ap = et[:]
                    pend.append((jb, e_ap))
                    if len(pend) > 3:
                        emit_av(*pend.pop(0))
                    if jb in (3, 7, 11) and filler is not None:
                        filler(jb // 4)
                for item in pend:
                    emit_av(*item)

                # denominator row to partition 0: psum -> sbuf same-lane,
                # then an sbuf->sbuf copy with a 64->0 partition shift (ACT
                # has slack in every sweep)
                den0 = p_dn.tile([1, 1024], F32, tag="d0")
                if drain:
                    # last sweep of the batch: move o^T out of psum right
                    # away (on ACT -- DVE must stay clear for the upcoming
                    # norm + stage-A copies) so projection reuses the banks
                    oTu = p_rc.tile([65, 1024], F32, tag="u")
                    for half in range(2):
                        nc.scalar.copy(
                            oTu[:, half * 512:(half + 1) * 512],
                            avps[half][:])
                    nc.scalar.copy(den0[0:1, :], oTu[64:65, :])
                    srcs = [oTu[0:64, 0:512], oTu[0:64, 512:1024]]
                else:
                    den = p_dn.tile([65, 1024], F32, tag="dn")
                    for half in range(2):
                        nc.scalar.copy(
                            den[64:65, half * 512:(half + 1) * 512],
                            avps[half][64:65, :])
                    nc.scalar.copy(den0[0:1, :], den[64:65, :])

                state = {}

                def norm_step(step):
                    # staged so the DVE filler ops interleave between
                    # fast-exps instead of jamming the queue mid-sweep
                    if step == 0:
                        rcp = p_dn.tile([1, 1024], F32, tag="rc")
                        nc.vector.reciprocal_approx_fast(
                            rcp[0:1, :], den0[0:1, :])
                        bcs = p_bc.tile([64, 1024], F32, tag="bc")
                        nc.gpsimd.partition_broadcast(bcs[:], rcp[0:1, :])
                        state["bcs"] = bcs
                    else:
                        half = step - 1
                        src = srcs[half] if drain else avps[half][0:64, :]
                        nc.vector.tensor_mul(
                            oTn[h * 64:(h + 1) * 64,
                                q0 + half * 512:q0 + (half + 1) * 512],
                            src,
                            state["bcs"][:, half * 512:(half + 1) * 512])
                return norm_step

            emit_x_loads(0)
            wq_sb = p_w.tile([128, 3072], BF16)
            nc.sync.dma_start(wq_sb[:], wqkv[:])
            wp_sb = p_w.tile([128, D], BF16)
            nc.sync.dma_start(wp_sb[:], wproj[:])
            proj_prev = None
            for b in range(B):
                qt, kt, vON2 = emit_stage_a(b, proj_prev)
                if b + 1 < B:
                    emit_x_loads(b + 1)
                oTn = p_on.tile([128, P], BF16, tag="on", name=f"oTn{b}")
                fn = None
                for si, (h, qc) in enumerate([(0, 0), (0, 1), (1, 0), (1, 1)]):
                    fn = emit_sweep(qt, kt, vON2, oTn, h, qc, filler=fn,
                                    drain=(si == 3))
                for stp in range(3):
                    fn(stp)
                proj_prev = (b, oTn)
            # last batch's projection has no following stage A
            for rr in range(16):
                emit_proj_unit(B - 1, rr, proj_prev[1])

    nc.compile()
    return nc


def _in_maps(x, w_qkv, w_proj):
    x2 = x.reshape(R, D).T                                # (D, R)
    # pre-tile to the device DMA layout: [p, c, kb, n] = x2[kb*128+p, c*512+n]
    xbf = np.ascontiguousarray(
        x2.reshape(8, 128, 16, 512).transpose(1, 2, 0, 3)
    ).reshape(128, 16 * 8 * 512).astype(ml_dtypes.bfloat16)
    Wq = w_qkv.reshape(D, 3, H, d)
    Wp = w_proj.reshape(H, d, D)
    maps = []
    for c in range(NCORES):
        hs = slice(c * HPC, (c + 1) * HPC)
        # per-core qkv weight shard, columns ordered (qkv, head, d)
        w_shard = np.ascontiguousarray(Wq[:, :, hs, :]).reshape(D, 3 * HPC * d)
        # pre-tile: [p, kb*384 + m*128 + col] = w_shard[kb*128+p, m*128+col]
        wq_pre = np.ascontiguousarray(
            w_shard.reshape(8, 128, 3, 128).transpose(1, 0, 2, 3)
        ).reshape(128, 3072)
        wp_shard = np.ascontiguousarray(Wp[hs]).reshape(HPC * d, D)
        maps.append({
            "xT": xbf,
            "wqkv": np.ascontiguousarray(wq_pre).astype(ml_dtypes.bfloat16),
            "wproj": wp_shard.astype(ml_dtypes.bfloat16),
        })
    return maps


def get_nc():
    if "nc" not in _CACHE:
        _CACHE["nc"] = _build()
    return _CACHE["nc"]


def kernel(x, w_qkv, w_proj, b_proj):
    x = np.asarray(x)
    w_qkv = np.asarray(w_qkv)
    w_proj = np.asarray(w_proj)
    b_proj = np.asarray(b_proj)
    nc = get_nc()
    maps = _in_maps(x, w_qkv, w_proj)
    res = bass_utils.run_bass_kernel_spmd(nc, maps, core_ids=list(range(NCORES)))
    acc = np.zeros((R, D), dtype=np.float64)
    for r in res.results:
        acc += r["out"].astype(np.float64)
    acc += b_proj.astype(np.float64)
    return acc.reshape(B, P, D).astype(np.float32)


# revision 33
# speedup vs baseline: 1.1294x; 1.0166x over previous
"""Multi-head attention (B=4, P=2048, D=1024, H=16) on 8 TRN2 NeuronCores.

Sharding: tensor-parallel over heads (2 heads per core). Each core computes
qkv for its heads, full attention for its heads, and a partial output
projection (rows of w_proj for its heads). Partials are summed on host.

v7: PE-paced pipeline, every engine under the PE budget per sweep so the
tensor engine never idles (idle gaps reset its p-state and halve its
clock). Per key-block the PE does 2 score + 2 attention-value matmuls
(864ns @2.4GHz); exps alternate between ACT (even blocks, 1147ns) and DVE
fast-exp (odd blocks, 1223ns: affine to int16 bits reinterpreted as bf16),
so each exp engine runs one op per 1728ns budget with slack and no
back-to-back drift. Attention-value matmuls lag scores by 2 blocks to hide
exp latency. Softmax denominators ride a ones-column in the [v|1]
stationary; normalization runs entirely off the PE: ACT copies the
denominator row out of psum, DVE takes a fast reciprocal, the idle GpSimd
engine broadcasts it across partitions, and DVE multiplies straight out of
the attention psum -- scheduled one sweep later so nothing ever waits.
Output projection uses K=128 (both heads' dims stacked on partitions);
V transposes run on the PE (53ns each) from a staging copy.
"""

import numpy as np
import ml_dtypes

import concourse.bass as bass
import concourse.tile as tile
from concourse import bacc, mybir
from concourse import bass_utils
from concourse.masks import make_identity

B, P, D = 4, 2048, 1024
H = 16
NCORES = 8
HPC = H // NCORES          # heads per core = 2
d = D // H                 # 64
R = B * P                  # 8192
SCALE = float(d) ** -0.5

F32 = mybir.dt.float32
BF16 = mybir.dt.bfloat16
I16 = mybir.dt.int16
AF = mybir.ActivationFunctionType
ALU = mybir.AluOpType

# fast-exp: bf16 bits of exp(s*SCALE) ~= round(s*KMUL + BADD)
KMUL = SCALE * float(np.log2(np.e)) * 128.0
BADD = 127.0 * 128.0 - 7.5

_CACHE = {}


def _build():
    nc = bacc.Bacc("TRN2", target_bir_lowering=False, debug=False,
                   enable_asserts=False)
    xT = nc.dram_tensor("xT", (128, 16 * 8 * 512), BF16,
                        kind="ExternalInput").ap()
    wqkv = nc.dram_tensor("wqkv", (128, 3072), BF16, kind="ExternalInput").ap()
    wproj = nc.dram_tensor("wproj", (128, D), BF16, kind="ExternalInput").ap()
    out = nc.dram_tensor("out", (R, D), F32, kind="ExternalOutput").ap()

    xT4 = xT.rearrange("p (c kb n) -> p c kb n", kb=8, n=512)
    out3 = out.rearrange("(r p) n -> p r n", p=128)      # [128, 64, 1024]

    with tile.TileContext(nc) as tc:
        from contextlib import ExitStack
        with ExitStack() as ctx:
            p_const = ctx.enter_context(tc.tile_pool(name="const", bufs=1))
            p_w = ctx.enter_context(tc.tile_pool(name="w", bufs=1))
            p_x = ctx.enter_context(tc.tile_pool(name="x", bufs=8))
            p_qk = ctx.enter_context(tc.tile_pool(name="qk", bufs=2))
            p_vt = ctx.enter_context(tc.tile_pool(name="vt", bufs=2))
            p_v = ctx.enter_context(tc.tile_pool(name="v", bufs=2))
            p_e = ctx.enter_context(tc.tile_pool(name="e", bufs=4))
            p_ei = ctx.enter_context(tc.tile_pool(name="ei", bufs=4))
            p_dn = ctx.enter_context(tc.tile_pool(name="dn", bufs=2))
            p_rc = ctx.enter_context(tc.tile_pool(name="rc", bufs=2))
            p_bc = ctx.enter_context(tc.tile_pool(name="bc", bufs=2))
            p_on = ctx.enter_context(tc.tile_pool(name="on", bufs=2))
            p_out = ctx.enter_context(tc.tile_pool(name="o", bufs=3))
            # PSUM: ss 2x[128,1024]f32 (4 banks) + av 4x[*,512] (4) = 8
            ps_ss = ctx.enter_context(
                tc.tile_pool(name="pss", bufs=2, space="PSUM"))
            ps_av = ctx.enter_context(
                tc.tile_pool(name="psav", bufs=4, space="PSUM"))

            ident = p_const.tile([128, 128], BF16)
            make_identity(nc, ident[:])

            xts = {}

            def emit_x_loads(b, ccs=range(4)):
                for cc in ccs:
                    c = b * 4 + cc
                    xt = p_x.tile([128, 8 * 512], BF16, tag="x")
                    nc.sync.dma_start(
                        xt.rearrange("p (kb n) -> p kb n", n=512),
                        xT4[:, c, :, :])
                    xts[(b, cc)] = xt

            def emit_proj_unit(b, rr, oTn):
                """one token-block of the output projection (2 matmuls into
                the [128,512] av-ring + copies split ACT/DVE + store)"""
                pa = ps_av.tile([128, 512], F32, tag="av", name="pPa")
                pb = ps_av.tile([128, 512], F32, tag="av", name="pPb")
                nc.tensor.matmul(pa[:], oTn[:, rr * 128:(rr + 1) * 128],
                                 wp_sb[:, 0:512], start=True, stop=True)
                nc.tensor.matmul(pb[:], oTn[:, rr * 128:(rr + 1) * 128],
                                 wp_sb[:, 512:1024], start=True, stop=True)
                outsb = p_out.tile([128, 1024], F32, tag="os")
                if rr % 2 == 0:
                    nc.scalar.copy(outsb[:, 0:512], pa[:])
                    nc.scalar.copy(outsb[:, 512:1024], pb[:])
                else:
                    nc.vector.tensor_copy(outsb[:, 0:512], pa[:])
                    nc.vector.tensor_copy(outsb[:, 512:1024], pb[:])
                r0 = b * 16 + rr
                nc.sync.dma_start(
                    out3[:, r0:r0 + 1, :],
                    outsb.rearrange("p (r n) -> p r n", n=1024))

            def emit_stage_a(b, proj_prev=None):
                """qkv for batch b -> qt, kt, vON2. Interleaves the previous
                batch's output projection (4 token-blocks per cc) so its
                psum copies spread across the stage instead of jamming the
                ACT/DVE queues at a phase boundary."""
                qt = p_qk.tile([128, P], BF16, tag="qt", name=f"qt{b}")
                kt = p_qk.tile([128, P], BF16, tag="kt", name=f"kt{b}")
                vON2 = p_v.tile([128, 16 * 130], BF16, tag="v",
                                name=f"vON{b}")
                vv = vON2.rearrange("p (blk w) -> p blk w", w=130)
                nc.vector.memset(vv[:, :, 64:65], 1.0)
                nc.vector.memset(vv[:, :, 129:130], 1.0)
                for cc in range(4):
                    xt = xts.pop((b, cc))
                    # v first: its staging copy drains while the q/k matmuls
                    # run, so the transposes at the end never wait
                    psB = ps_ss.tile([128, 1024], F32, tag="ss", name="psB")
                    for kb in range(8):
                        col = kb * 384 + 256
                        nc.tensor.matmul(
                            psB[:, 0:512], wq_sb[:, col:col + 128],
                            xt[:, kb * 512:(kb + 1) * 512],
                            start=(kb == 0), stop=(kb == 7))
                    vtmp = p_vt.tile([128, 512], BF16, tag="vt")
                    nc.vector.tensor_copy(vtmp[:], psB[:, 0:512])
                    psA = ps_ss.tile([128, 1024], F32, tag="ss", name="psA")
                    for m in range(2):
                        for kb in range(8):
                            col = kb * 384 + m * 128
                            nc.tensor.matmul(
                                psA[:, m * 512:(m + 1) * 512],
                                wq_sb[:, col:col + 128],
                                xt[:, kb * 512:(kb + 1) * 512],
                                start=(kb == 0), stop=(kb == 7))
                    psT = ps_av.tile([128, 512], BF16, tag="av", name="psT")
                    for i in range(4):
                        nc.tensor.transpose(
                            psT[:, i * 128:(i + 1) * 128],
                            vtmp[:, i * 128:(i + 1) * 128], ident[:])
                    nc.vector.tensor_copy(
                        qt[:, cc * 512:(cc + 1) * 512], psA[:, 0:512])
                    nc.vector.tensor_copy(
                        kt[:, cc * 512:(cc + 1) * 512], psA[:, 512:1024])
                    for i in range(4):
                        jb = cc * 4 + i
                        dstv = vv[:, jb, 0:130].rearrange(
                            "p (two s) -> p two s", s=65)[:, :, 0:64]
                        srcv = psT[:, i * 128:(i + 1) * 128].rearrange(
                            "p (two s) -> p two s", s=64)
                        nc.vector.tensor_copy(dstv, srcv)
                    if proj_prev is not None:
                        pb_, oTn_ = proj_prev
                        for rr in range(cc * 4, cc * 4 + 4):
                            emit_proj_unit(pb_, rr, oTn_)
                return qt, kt, vON2

            def emit_sweep(qt, kt, vON2, oTn, h, qc, filler=None, drain=False):
                """scores+exp+av for one (head, 1024-query chunk).

                Returns a closure finishing this sweep's normalization; the
                caller passes it as `filler` into the NEXT sweep (emitted
                after key-block 3) so every input is long since ready.
                """
                q0 = qc * 1024
                avps = [ps_av.tile([65, 512], F32, tag="av", name="avps")
                        for _ in range(2)]
                vs = vON2.rearrange("p (blk w) -> p blk w", w=130)
                pend = []

                def emit_av(jb, e_ap):
                    for half in range(2):
                        nc.tensor.matmul(
                            avps[half][:],
                            vs[:, jb, h * 65:(h + 1) * 65],
                            e_ap[:, half * 512:(half + 1) * 512],
                            start=(jb == 0), stop=(jb == 15))

                for jb in range(16):
                    pss = ps_ss.tile([128, 1024], F32, tag="ss", name="pss")
                    for half in range(2):
                        nc.tensor.matmul(
                            pss[:, half * 512:(half + 1) * 512],
                            kt[h * 64:(h + 1) * 64, jb * 128:(jb + 1) * 128],
                            qt[h * 64:(h + 1) * 64,
                               q0 + half * 512:q0 + (half + 1) * 512],
                            start=True, stop=True)
                    if jb % 2 == 1:
                        ei = p_ei.tile([128, 1024], I16, tag="ei")
                        nc.vector.tensor_scalar(ei[:], pss[:], KMUL, BADD,
                                                ALU.mult, ALU.add)
                        e_ap = ei[:].bitcast(BF16)
                    else:
                        et = p_e.tile([128, 1024], BF16, tag="e")
                        nc.scalar.activation(et[:], pss[:], AF.Exp,
                                             scale=SCALE)
                        e_ap = et[:]
                    pend.append((jb, e_ap))
                    if len(pend) > 3:
                        emit_av(*pend.pop(0))
                    if jb in (3, 7, 11) and filler is not None:
                        filler(jb // 4)
                for item in pend:
                    emit_av(*item)

                # denominator row to partition 0: psum -> sbuf same-lane,
                # then an sbuf->sbuf copy with a 64->0 partition shift (ACT
                # has slack in every sweep)
                den0 = p_dn.tile([1, 1024], F32, tag="d0")
                if drain:
                    # last sweep of the batch: move o^T out of psum right
                    # away (on ACT -- DVE must stay clear for the upcoming
                    # norm + stage-A copies) so projection reuses the banks
                    oTu = p_rc.tile([65, 1024], F32, tag="u")
                    for half in range(2):
                        nc.scalar.copy(
                            oTu[:, half * 512:(half + 1) * 512],
                            avps[half][:])
                    nc.scalar.copy(den0[0:1, :], oTu[64:65, :])
                    srcs = [oTu[0:64, 0:512], oTu[0:64, 512:1024]]
                else:
                    den = p_dn.tile([65, 1024], F32, tag="dn")
                    for half in range(2):
                        nc.scalar.copy(
                            den[64:65, half * 512:(half + 1) * 512],
                            avps[half][64:65, :])
                    nc.scalar.copy(den0[0:1, :], den[64:65, :])

                state = {}

                def norm_step(step):
                    # staged so the DVE filler ops interleave between
                    # fast-exps instead of jamming the queue mid-sweep
                    if step == 0:
                        rcp = p_dn.tile([1, 1024], F32, tag="rc")
                        nc.vector.reciprocal_approx_fast(
                            rcp[0:1, :], den0[0:1, :])
                        bcs = p_bc.tile([64, 1024], F32, tag="bc")
                        nc.gpsimd.partition_broadcast(bcs[:], rcp[0:1, :])
                        state["bcs"] = bcs
                    else:
                        half = step - 1
                        src = srcs[half] if drain else avps[half][0:64, :]
                        nc.vector.tensor_mul(
                            oTn[h * 64:(h + 1) * 64,
                                q0 + half * 512:q0 + (half + 1) * 512],
                            src,
                            state["bcs"][:, half * 512:(half + 1) * 512])
                return norm_step

            # load order: cc0's x and the qkv weights race to the first
            # matmul; everything else queues behind them
            emit_x_loads(0, ccs=[0])
            wq_sb = p_w.tile([128, 3072], BF16)
            nc.sync.dma_start(wq_sb[:], wqkv[:])
            wp_sb = p_w.tile([128, D], BF16)
            nc.sync.dma_start(wp_sb[:], wproj[:])
            emit_x_loads(0, ccs=[1, 2, 3])
            proj_prev = None
            for b in range(B):
                qt, kt, vON2 = emit_stage_a(b, proj_prev)
                if b + 1 < B:
                    emit_x_loads(b + 1)
                oTn = p_on.tile([128, P], BF16, tag="on", name=f"oTn{b}")
                fn = None
                for si, (h, qc) in enumerate([(0, 0), (0, 1), (1, 0), (1, 1)]):
                    fn = emit_sweep(qt, kt, vON2, oTn, h, qc, filler=fn,
                                    drain=(si == 3))
                for stp in range(3):
                    fn(stp)
                proj_prev = (b, oTn)
            # last batch's projection has no following stage A
            for rr in range(16):
                emit_proj_unit(B - 1, rr, proj_prev[1])

    nc.compile()
    return nc


def _in_maps(x, w_qkv, w_proj):
    x2 = x.reshape(R, D).T                                # (D, R)
    # pre-tile to the device DMA layout: [p, c, kb, n] = x2[kb*128+p, c*512+n]
    xbf = np.ascontiguousarray(
        x2.reshape(8, 128, 16, 512).transpose(1, 2, 0, 3)
    ).reshape(128, 16 * 8 * 512).astype(ml_dtypes.bfloat16)
    Wq = w_qkv.reshape(D, 3, H, d)
    Wp = w_proj.reshape(H, d, D)
    maps = []
    for c in range(NCORES):
        hs = slice(c * HPC, (c + 1) * HPC)
        # per-core qkv weight shard, columns ordered (qkv, head, d)
        w_shard = np.ascontiguousarray(Wq[:, :, hs, :]).reshape(D, 3 * HPC * d)
        # pre-tile: [p, kb*384 + m*128 + col] = w_shard[kb*128+p, m*128+col]
        wq_pre = np.ascontiguousarray(
            w_shard.reshape(8, 128, 3, 128).transpose(1, 0, 2, 3)
        ).reshape(128, 3072)
        wp_shard = np.ascontiguousarray(Wp[hs]).reshape(HPC * d, D)
        maps.append({
            "xT": xbf,
            "wqkv": np.ascontiguousarray(wq_pre).astype(ml_dtypes.bfloat16),
            "wproj": wp_shard.astype(ml_dtypes.bfloat16),
        })
    return maps


def get_nc():
    if "nc" not in _CACHE:
        _CACHE["nc"] = _build()
    return _CACHE["nc"]


def kernel(x, w_qkv, w_proj, b_proj):
    x = np.asarray(x)
    w_qkv = np.asarray(w_qkv)
    w_proj = np.asarray(w_proj)
    b_proj = np.asarray(b_proj)
    nc = get_nc()
    maps = _in_maps(x, w_qkv, w_proj)
    res = bass_utils.run_bass_kernel_spmd(nc, maps, core_ids=list(range(NCORES)))
    acc = np.zeros((R, D), dtype=np.float64)
    for r in res.results:
        acc += r["out"].astype(np.float64)
    acc += b_proj.astype(np.float64)
    return acc.reshape(B, P, D).astype(np.float32)


# revision 34
# speedup vs baseline: 1.1639x; 1.0305x over previous
"""Multi-head attention (B=4, P=2048, D=1024, H=16) on 8 TRN2 NeuronCores.

Sharding: tensor-parallel over heads (2 heads per core). Each core computes
qkv for its heads, full attention for its heads, and a partial output
projection (rows of w_proj for its heads). Partials are summed on host.

v7: PE-paced pipeline, every engine under the PE budget per sweep so the
tensor engine never idles (idle gaps reset its p-state and halve its
clock). Per key-block the PE does 2 score + 2 attention-value matmuls
(864ns @2.4GHz); exps alternate between ACT (even blocks, 1147ns) and DVE
fast-exp (odd blocks, 1223ns: affine to int16 bits reinterpreted as bf16),
so each exp engine runs one op per 1728ns budget with slack and no
back-to-back drift. Attention-value matmuls lag scores by 2 blocks to hide
exp latency. Softmax denominators ride a ones-column in the [v|1]
stationary; normalization runs entirely off the PE: ACT copies the
denominator row out of psum, DVE takes a fast reciprocal, the idle GpSimd
engine broadcasts it across partitions, and DVE multiplies straight out of
the attention psum -- scheduled one sweep later so nothing ever waits.
Output projection uses K=128 (both heads' dims stacked on partitions);
V transposes run on the PE (53ns each) from a staging copy.
"""

import numpy as np
import ml_dtypes

import concourse.bass as bass
import concourse.tile as tile
from concourse import bacc, mybir
from concourse import bass_utils
from concourse.masks import make_identity

B, P, D = 4, 2048, 1024
H = 16
NCORES = 8
HPC = H // NCORES          # heads per core = 2
d = D // H                 # 64
R = B * P                  # 8192
SCALE = float(d) ** -0.5

F32 = mybir.dt.float32
BF16 = mybir.dt.bfloat16
I16 = mybir.dt.int16
AF = mybir.ActivationFunctionType
ALU = mybir.AluOpType

# fast-exp: bf16 bits of exp(s*SCALE) ~= round(s*KMUL + BADD)
KMUL = SCALE * float(np.log2(np.e)) * 128.0
BADD = 127.0 * 128.0 - 7.5

_CACHE = {}


def _build():
    nc = bacc.Bacc("TRN2", target_bir_lowering=False, debug=False,
                   enable_asserts=False)
    xT = nc.dram_tensor("xT", (128, 16 * 8 * 512), BF16,
                        kind="ExternalInput").ap()
    wqkv = nc.dram_tensor("wqkv", (128, 3072), BF16, kind="ExternalInput").ap()
    wproj = nc.dram_tensor("wproj", (128, D), BF16, kind="ExternalInput").ap()
    out = nc.dram_tensor("out", (R, D), F32, kind="ExternalOutput").ap()

    xT4 = xT.rearrange("p (c kb n) -> p c kb n", kb=8, n=512)
    out3 = out.rearrange("(r p) n -> p r n", p=128)      # [128, 64, 1024]

    with tile.TileContext(nc) as tc:
        from contextlib import ExitStack
        with ExitStack() as ctx:
            p_const = ctx.enter_context(tc.tile_pool(name="const", bufs=1))
            p_w = ctx.enter_context(tc.tile_pool(name="w", bufs=1))
            p_x = ctx.enter_context(tc.tile_pool(name="x", bufs=8))
            p_qk = ctx.enter_context(tc.tile_pool(name="qk", bufs=2))
            p_vt = ctx.enter_context(tc.tile_pool(name="vt", bufs=2))
            p_v = ctx.enter_context(tc.tile_pool(name="v", bufs=2))
            p_e = ctx.enter_context(tc.tile_pool(name="e", bufs=4))
            p_ei = ctx.enter_context(tc.tile_pool(name="ei", bufs=4))
            p_dn = ctx.enter_context(tc.tile_pool(name="dn", bufs=2))
            p_rc = ctx.enter_context(tc.tile_pool(name="rc", bufs=2))
            p_bc = ctx.enter_context(tc.tile_pool(name="bc", bufs=2))
            p_on = ctx.enter_context(tc.tile_pool(name="on", bufs=2))
            p_out = ctx.enter_context(tc.tile_pool(name="o", bufs=3))
            # PSUM: ss 2x[128,1024]f32 (4 banks) + av 4x[*,512] (4) = 8
            ps_ss = ctx.enter_context(
                tc.tile_pool(name="pss", bufs=2, space="PSUM"))
            ps_av = ctx.enter_context(
                tc.tile_pool(name="psav", bufs=4, space="PSUM"))

            ident = p_const.tile([128, 128], BF16)
            make_identity(nc, ident[:])

            xts = {}

            def emit_x_loads(b, ccs=range(4)):
                for cc in ccs:
                    c = b * 4 + cc
                    xt = p_x.tile([128, 8 * 512], BF16, tag="x")
                    nc.sync.dma_start(
                        xt.rearrange("p (kb n) -> p kb n", n=512),
                        xT4[:, c, :, :])
                    xts[(b, cc)] = xt

            def emit_proj_unit(b, rr, oTn):
                """one token-block of the output projection (2 matmuls into
                the [128,512] av-ring + copies split ACT/DVE + store)"""
                pa = ps_av.tile([128, 512], F32, tag="av", name="pPa")
                pb = ps_av.tile([128, 512], F32, tag="av", name="pPb")
                nc.tensor.matmul(pa[:], oTn[:, rr * 128:(rr + 1) * 128],
                                 wp_sb[:, 0:512], start=True, stop=True)
                nc.tensor.matmul(pb[:], oTn[:, rr * 128:(rr + 1) * 128],
                                 wp_sb[:, 512:1024], start=True, stop=True)
                outsb = p_out.tile([128, 1024], F32, tag="os")
                if rr % 2 == 0:
                    nc.scalar.copy(outsb[:, 0:512], pa[:])
                    nc.scalar.copy(outsb[:, 512:1024], pb[:])
                else:
                    nc.vector.tensor_copy(outsb[:, 0:512], pa[:])
                    nc.vector.tensor_copy(outsb[:, 512:1024], pb[:])
                r0 = b * 16 + rr
                nc.sync.dma_start(
                    out3[:, r0:r0 + 1, :],
                    outsb.rearrange("p (r n) -> p r n", n=1024))

            def emit_stage_a(b, proj_prev=None):
                """qkv for batch b -> qt, kt, vON2. Interleaves the previous
                batch's output projection (4 token-blocks per cc) so its
                psum copies spread across the stage instead of jamming the
                ACT/DVE queues at a phase boundary."""
                qt = p_qk.tile([128, P], BF16, tag="qt", name=f"qt{b}")
                kt = p_qk.tile([128, P], BF16, tag="kt", name=f"kt{b}")
                vON2 = p_v.tile([128, 16 * 130], BF16, tag="v",
                                name=f"vON{b}")
                vv = vON2.rearrange("p (blk w) -> p blk w", w=130)
                nc.vector.memset(vv[:, :, 64:65], 1.0)
                nc.vector.memset(vv[:, :, 129:130], 1.0)
                pend_T = None
                for cc in range(4):
                    xt = xts.pop((b, cc))
                    # v first: its staging copy drains while the q/k matmuls
                    # run, so the transposes at the end never wait
                    psB = ps_ss.tile([128, 1024], F32, tag="ss", name="psB")
                    for kb in range(8):
                        col = kb * 384 + 256
                        nc.tensor.matmul(
                            psB[:, 0:512], wq_sb[:, col:col + 128],
                            xt[:, kb * 512:(kb + 1) * 512],
                            start=(kb == 0), stop=(kb == 7))
                    vtmp = p_vt.tile([128, 512], BF16, tag="vt")
                    nc.vector.tensor_copy(vtmp[:], psB[:, 0:512])
                    psA = ps_ss.tile([128, 1024], F32, tag="ss", name="psA")
                    for m in range(2):
                        for kb in range(8):
                            col = kb * 384 + m * 128
                            nc.tensor.matmul(
                                psA[:, m * 512:(m + 1) * 512],
                                wq_sb[:, col:col + 128],
                                xt[:, kb * 512:(kb + 1) * 512],
                                start=(kb == 0), stop=(kb == 7))
                    def emit_T(vtmp_, cc_):
                        psT = ps_av.tile([128, 512], BF16, tag="av",
                                         name="psT")
                        for i in range(4):
                            nc.tensor.transpose(
                                psT[:, i * 128:(i + 1) * 128],
                                vtmp_[:, i * 128:(i + 1) * 128], ident[:])
                        for i in range(4):
                            jb = cc_ * 4 + i
                            dstv = vv[:, jb, 0:130].rearrange(
                                "p (two s) -> p two s", s=65)[:, :, 0:64]
                            srcv = psT[:, i * 128:(i + 1) * 128].rearrange(
                                "p (two s) -> p two s", s=64)
                            nc.vector.tensor_copy(dstv, srcv)

                    if b == 0:
                        # cold-start: the DVE ramps slowly, so give the
                        # vtmp staging copy a full cc of slack before the
                        # transposes consume it (av-ring is empty here)
                        if pend_T is not None:
                            emit_T(*pend_T)
                        pend_T = (vtmp, cc)
                    else:
                        emit_T(vtmp, cc)
                    nc.vector.tensor_copy(
                        qt[:, cc * 512:(cc + 1) * 512], psA[:, 0:512])
                    nc.vector.tensor_copy(
                        kt[:, cc * 512:(cc + 1) * 512], psA[:, 512:1024])
                    if proj_prev is not None:
                        pb_, oTn_ = proj_prev
                        for rr in range(cc * 4, cc * 4 + 4):
                            emit_proj_unit(pb_, rr, oTn_)
                if pend_T is not None:
                    emit_T(*pend_T)
                return qt, kt, vON2

            def emit_sweep(qt, kt, vON2, oTn, h, qc, filler=None, drain=False):
                """scores+exp+av for one (head, 1024-query chunk).

                Returns a closure finishing this sweep's normalization; the
                caller passes it as `filler` into the NEXT sweep (emitted
                after key-block 3) so every input is long since ready.
                """
                q0 = qc * 1024
                avps = [ps_av.tile([65, 512], F32, tag="av", name="avps")
                        for _ in range(2)]
                vs = vON2.rearrange("p (blk w) -> p blk w", w=130)
                pend = []

                def emit_av(jb, e_ap):
                    for half in range(2):
                        nc.tensor.matmul(
                            avps[half][:],
                            vs[:, jb, h * 65:(h + 1) * 65],
                            e_ap[:, half * 512:(half + 1) * 512],
                            start=(jb == 0), stop=(jb == 15))

                for jb in range(16):
                    pss = ps_ss.tile([128, 1024], F32, tag="ss", name="pss")
                    for half in range(2):
                        nc.tensor.matmul(
                            pss[:, half * 512:(half + 1) * 512],
                            kt[h * 64:(h + 1) * 64, jb * 128:(jb + 1) * 128],
                            qt[h * 64:(h + 1) * 64,
                               q0 + half * 512:q0 + (half + 1) * 512],
                            start=True, stop=True)
                    if jb % 2 == 1:
                        ei = p_ei.tile([128, 1024], I16, tag="ei")
                        nc.vector.tensor_scalar(ei[:], pss[:], KMUL, BADD,
                                                ALU.mult, ALU.add)
                        e_ap = ei[:].bitcast(BF16)
                    else:
                        et = p_e.tile([128, 1024], BF16, tag="e")
                        nc.scalar.activation(et[:], pss[:], AF.Exp,
                                             scale=SCALE)
                        e_ap = et[:]
                    pend.append((jb, e_ap))
                    if len(pend) > 3:
                        emit_av(*pend.pop(0))
                    if jb in (3, 7, 11) and filler is not None:
                        filler(jb // 4)
                for item in pend:
                    emit_av(*item)

                # denominator row to partition 0: psum -> sbuf same-lane,
                # then an sbuf->sbuf copy with a 64->0 partition shift (ACT
                # has slack in every sweep)
                den0 = p_dn.tile([1, 1024], F32, tag="d0")
                if drain:
                    # last sweep of the batch: move o^T out of psum right
                    # away (on ACT -- DVE must stay clear for the upcoming
                    # norm + stage-A copies) so projection reuses the banks
                    oTu = p_rc.tile([65, 1024], F32, tag="u")
                    for half in range(2):
                        nc.scalar.copy(
                            oTu[:, half * 512:(half + 1) * 512],
                            avps[half][:])
                    nc.scalar.copy(den0[0:1, :], oTu[64:65, :])
                    srcs = [oTu[0:64, 0:512], oTu[0:64, 512:1024]]
                else:
                    den = p_dn.tile([65, 1024], F32, tag="dn")
                    for half in range(2):
                        nc.scalar.copy(
                            den[64:65, half * 512:(half + 1) * 512],
                            avps[half][64:65, :])
                    nc.scalar.copy(den0[0:1, :], den[64:65, :])

                state = {}

                def norm_step(step):
                    # staged so the DVE filler ops interleave between
                    # fast-exps instead of jamming the queue mid-sweep
                    if step == 0:
                        rcp = p_dn.tile([1, 1024], F32, tag="rc")
                        nc.vector.reciprocal_approx_fast(
                            rcp[0:1, :], den0[0:1, :])
                        bcs = p_bc.tile([64, 1024], F32, tag="bc")
                        nc.gpsimd.partition_broadcast(bcs[:], rcp[0:1, :])
                        state["bcs"] = bcs
                    else:
                        half = step - 1
                        src = srcs[half] if drain else avps[half][0:64, :]
                        nc.vector.tensor_mul(
                            oTn[h * 64:(h + 1) * 64,
                                q0 + half * 512:q0 + (half + 1) * 512],
                            src,
                            state["bcs"][:, half * 512:(half + 1) * 512])
                return norm_step

            # load order: cc0's x and the qkv weights race to the first
            # matmul; everything else queues behind them
            emit_x_loads(0, ccs=[0])
            wq_sb = p_w.tile([128, 3072], BF16)
            nc.sync.dma_start(wq_sb[:], wqkv[:])
            wp_sb = p_w.tile([128, D], BF16)
            nc.sync.dma_start(wp_sb[:], wproj[:])
            emit_x_loads(0, ccs=[1, 2, 3])
            proj_prev = None
            for b in range(B):
                qt, kt, vON2 = emit_stage_a(b, proj_prev)
                if b + 1 < B:
                    emit_x_loads(b + 1)
                oTn = p_on.tile([128, P], BF16, tag="on", name=f"oTn{b}")
                fn = None
                for si, (h, qc) in enumerate([(0, 0), (0, 1), (1, 0), (1, 1)]):
                    fn = emit_sweep(qt, kt, vON2, oTn, h, qc, filler=fn,
                                    drain=(si == 3))
                for stp in range(3):
                    fn(stp)
                proj_prev = (b, oTn)
            # last batch's projection has no following stage A
            for rr in range(16):
                emit_proj_unit(B - 1, rr, proj_prev[1])

    nc.compile()
    return nc


def _in_maps(x, w_qkv, w_proj):
    x2 = x.reshape(R, D).T                                # (D, R)
    # pre-tile to the device DMA layout: [p, c, kb, n] = x2[kb*128+p, c*512+n]
    xbf = np.ascontiguousarray(
        x2.reshape(8, 128, 16, 512).transpose(1, 2, 0, 3)
    ).reshape(128, 16 * 8 * 512).astype(ml_dtypes.bfloat16)
    Wq = w_qkv.reshape(D, 3, H, d)
    Wp = w_proj.reshape(H, d, D)
    maps = []
    for c in range(NCORES):
        hs = slice(c * HPC, (c + 1) * HPC)
        # per-core qkv weight shard, columns ordered (qkv, head, d)
        w_shard = np.ascontiguousarray(Wq[:, :, hs, :]).reshape(D, 3 * HPC * d)
        # pre-tile: [p, kb*384 + m*128 + col] = w_shard[kb*128+p, m*128+col]
        wq_pre = np.ascontiguousarray(
            w_shard.reshape(8, 128, 3, 128).transpose(1, 0, 2, 3)
        ).reshape(128, 3072)
        wp_shard = np.ascontiguousarray(Wp[hs]).reshape(HPC * d, D)
        maps.append({
            "xT": xbf,
            "wqkv": np.ascontiguousarray(wq_pre).astype(ml_dtypes.bfloat16),
            "wproj": wp_shard.astype(ml_dtypes.bfloat16),
        })
    return maps


def get_nc():
    if "nc" not in _CACHE:
        _CACHE["nc"] = _build()
    return _CACHE["nc"]


def kernel(x, w_qkv, w_proj, b_proj):
    x = np.asarray(x)
    w_qkv = np.asarray(w_qkv)
    w_proj = np.asarray(w_proj)
    b_proj = np.asarray(b_proj)
    nc = get_nc()
    maps = _in_maps(x, w_qkv, w_proj)
    res = bass_utils.run_bass_kernel_spmd(nc, maps, core_ids=list(range(NCORES)))
    acc = np.zeros((R, D), dtype=np.float64)
    for r in res.results:
        acc += r["out"].astype(np.float64)
    acc += b_proj.astype(np.float64)
    return acc.reshape(B, P, D).astype(np.float32)
